# revision 6
# baseline (speedup 1.0000x reference)
"""Distributed Trainium2 kernel for CustomMultiHeadAttentionStoich.

Sharding: (batch, query-block) parallel over 8 cores — core c handles batch
c//4, queries [512*(c%4), 512*(c%4+1)).  Each core computes Q for its slice,
K/V for its whole batch (redundant 4x, no collectives), attention for all 16
heads over its 512 queries, and the output projection for its slice.  Host
concatenates the 8 disjoint output slices.

Key trick: the stoichiometric bias alpha_pos*relu(clamp(fk-fq)) +
alpha_neg*min(clamp(fk-fq),0) is a piecewise-linear kernel of (fk - fq); we
approximate it with a rank-16 SVD expansion per clamp half and inject the
feature maps as extra contraction rows of the Q@K^T matmul (head_dim=64
leaves 64 idle PE rows, so the bias costs zero extra PE cycles and zero
per-element vector work).

Scores are computed in [key, query] layout so softmax denominators come from
a ones-column appended to V (fused into the AV matmul), and the final
normalization happens on [64, 512] tiles before the output projection.
"""

import sys

sys.path.insert(0, "/opt/trn_rl_repo")

import numpy as np
import ml_dtypes

BF = ml_dtypes.bfloat16

B, T, D, H, DH = 2, 2048, 1024, 16, 64
NCORES = 8
TQ = 512  # queries per core
R = 16  # SVD rank per clamp-kernel half
AUG = 2 * R
CP = DH + AUG  # contraction rows for the scores matmul
NGRID = 1024  # SVD grid
KC = T // 128  # 16 key chunks
VG = 65  # V columns per head incl. ones column

_state = {}


def _features():
    """Rank-R SVD features of rc(x,y)=clip(x-y,0,0.2) on [0,1]^2."""
    if "grid" not in _state:
        g = (np.arange(NGRID) + 0.5) / NGRID
        M = np.clip(g[:, None] - g[None, :], 0.0, 0.2)
        U, S, Vt = np.linalg.svd(M, full_matrices=False)
        sc = np.sqrt(S[:R] * NGRID)
        _state["grid"] = g
        _state["phi"] = (U[:, :R] * sc).astype(np.float64)  # [NGRID, R] phi_j(x)
        _state["psi"] = (Vt[:R].T * sc).astype(np.float64)  # [NGRID, R] psi_j(y)
    return _state["grid"], _state["phi"], _state["psi"]


def _ev(tab, x):
    """Evaluate feature columns (linear interp) at points x -> [R, len(x)]."""
    g = _state["grid"]
    return np.stack([np.interp(x, g, tab[:, j]) for j in range(R)])


def _build():
    if "nc" in _state:
        return _state["nc"]

    import concourse.bass as bass
    import concourse.mybir as mybir
    import concourse.tile as tile
    from concourse import bacc

    dt = mybir.dt
    ts = bass.ts
    ds = bass.ds

    nc = bacc.Bacc(
        "TRN2",
        target_bir_lowering=False,
        debug=False,
        num_devices=NCORES,
    )

    # ---- kernel I/O ----
    xqT = nc.dram_tensor("xqT", [D, TQ], dt.bfloat16, kind="ExternalInput").ap()
    xkT = nc.dram_tensor("xkT", [D, T], dt.bfloat16, kind="ExternalInput").ap()
    xvT = nc.dram_tensor("xvT", [D, T], dt.bfloat16, kind="ExternalInput").ap()
    wqT = nc.dram_tensor("wqT", [D, D], dt.bfloat16, kind="ExternalInput").ap()
    wkT = nc.dram_tensor("wkT", [D, D], dt.bfloat16, kind="ExternalInput").ap()
    wvP = nc.dram_tensor("wvP", [D, H * VG], dt.bfloat16, kind="ExternalInput").ap()
    woT = nc.dram_tensor("woT", [D, D], dt.bfloat16, kind="ExternalInput").ap()
    bqE = nc.dram_tensor("bq", [D, 1], dt.float32, kind="ExternalInput").ap()
    bkE = nc.dram_tensor("bk", [D, 1], dt.float32, kind="ExternalInput").ap()
    bvA = nc.dram_tensor("bvA", [1, H * VG], dt.bfloat16, kind="ExternalInput").ap()
    boE = nc.dram_tensor("bo", [1, D], dt.bfloat16, kind="ExternalInput").ap()
    kfE = nc.dram_tensor("kfeat", [AUG, T], dt.bfloat16, kind="ExternalInput").ap()
    qfE = nc.dram_tensor("qfeat", [H * AUG, TQ], dt.bfloat16, kind="ExternalInput").ap()
    outE = nc.dram_tensor("out", [TQ, D], dt.float32, kind="ExternalOutput").ap()

    Exp = mybir.ActivationFunctionType.Exp
    Copy = mybir.ActivationFunctionType.Copy
    Ident = mybir.ActivationFunctionType.Identity

    with tile.TileContext(nc) as tc:
        with (
            tc.tile_pool(name="dram", bufs=1, space="DRAM") as dram,
            tc.tile_pool(name="consts", bufs=1) as consts,
            tc.tile_pool(name="xk", bufs=2) as xkp,
            tc.tile_pool(name="xv", bufs=3) as xvp,
            tc.tile_pool(name="kat", bufs=2) as katp,
            tc.tile_pool(name="vat", bufs=4) as vatp,
            tc.tile_pool(name="ehat", bufs=3) as ep,
            tc.tile_pool(name="stage", bufs=4) as stp,
            tc.tile_pool(name="den", bufs=2) as denp,
            tc.tile_pool(name="rsb", bufs=2) as rsbp,
            tc.tile_pool(name="ystage", bufs=2) as ysp,
            tc.tile_pool(name="psA", bufs=2, space="PSUM") as psA,
            tc.tile_pool(name="psS", bufs=2, space="PSUM") as psS,
            tc.tile_pool(name="psAV", bufs=2, space="PSUM") as psAV,
            tc.tile_pool(name="psR", bufs=1, space="PSUM") as psR,
        ):
            # DRAM intermediates
            kT_d = dram.tile([D, T], dt.bfloat16, tag="kT_d", name="kT_d")
            v_d = dram.tile([T, H * VG], dt.bfloat16, tag="v_d", name="v_d")

            # ---- resident constants ----
            wq_sb = consts.tile([128, 8, D], dt.bfloat16, tag="wq", name="wq")
            wk_sb = consts.tile([128, 8, D], dt.bfloat16, tag="wk", name="wk")
            wv_sb = consts.tile([128, 8, H * VG], dt.bfloat16, tag="wv", name="wv")
            # per-head Wo^T rows so every matmul operand starts at partition 0
            wo_sb = [
                consts.tile([DH, D], dt.bfloat16, tag=f"wo{h}", name=f"wo{h}") for h in range(H)
            ]
            xq_sb = consts.tile([128, 8, TQ], dt.bfloat16, tag="xq", name="xq")
            bq_sb = consts.tile([128, 8, 1], dt.float32, tag="bq", name="bq")
            bk_sb = consts.tile([128, 8, 1], dt.float32, tag="bk", name="bk")
            bv_sb = consts.tile([1, H * VG], dt.bfloat16, tag="bv", name="bv")
            bo_sb = consts.tile([1, D], dt.bfloat16, tag="bo", name="bo")
            ones_sb = consts.tile([1, 128], dt.bfloat16, tag="ones", name="ones")
            dsum = consts.tile([H, TQ], dt.float32, tag="dsum", name="dsum")
            rcp = consts.tile([H, TQ], dt.float32, tag="rcp", name="rcp")
            rcp_bf = consts.tile([H, TQ], dt.bfloat16, tag="rcpbf", name="rcpbf")

            for (dst, src) in (
                (wq_sb, wqT),
                (wk_sb, wkT),
                (xq_sb, xqT),
            ):
                nc.sync.dma_start(out=dst, in_=src.rearrange("(a p) m -> p a m", p=128))
            for h in range(H):
                nc.sync.dma_start(out=wo_sb[h], in_=woT[ds(DH * h, DH), :])
            nc.sync.dma_start(out=wv_sb, in_=wvP.rearrange("(a p) m -> p a m", p=128))
            nc.sync.dma_start(out=bq_sb, in_=bqE.rearrange("(a p) o -> p a o", p=128))
            nc.sync.dma_start(out=bk_sb, in_=bkE.rearrange("(a p) o -> p a o", p=128))
            nc.sync.dma_start(out=bv_sb, in_=bvA)
            nc.sync.dma_start(out=bo_sb, in_=boE)
            nc.vector.memset(ones_sb, 1.0)

            # per-head Q tiles [CP, TQ]: rows 0:64 Q^T head, 64:96 features
            qat = [consts.tile([CP, TQ], dt.bfloat16, tag=f"qat{h}", name=f"qat{h}") for h in range(H)]
            # per-head attention-out tiles [64, TQ] (unnormalized then normalized)
            aot = [consts.tile([DH, TQ], dt.bfloat16, tag=f"aot{h}", name=f"aot{h}") for h in range(H)]

            # ---- P1: Q projection ----
            for dc in range(8):
                ps = psA.tile([128, TQ], dt.float32, tag="mm", name="mm")
                for kc in range(8):
                    nc.tensor.matmul(
                        ps,
                        lhsT=wq_sb[:, kc, ts(dc, 128)],
                        rhs=xq_sb[:, kc, :],
                        start=(kc == 0),
                        stop=(kc == 7),
                    )
                stg = stp.tile([128, TQ], dt.bfloat16, tag="qstage", name="qstage")
                nc.scalar.activation(stg, ps, Ident, bias=bq_sb[:, dc, :])
                nc.sync.dma_start(out=qat[2 * dc][0:DH, :], in_=stg[0:DH, :])
                nc.sync.dma_start(out=qat[2 * dc + 1][0:DH, :], in_=stg[DH:128, :])
            for h in range(H):
                nc.sync.dma_start(out=qat[h][DH:CP, :], in_=qfE[ds(AUG * h, AUG), :])

            # ---- P2: K^T projection (full batch) -> kT_d ----
            for tc_i in range(4):
                xk_sb = xkp.tile([128, 8, 512], dt.bfloat16, tag="xk", name="xk")
                nc.sync.dma_start(
                    out=xk_sb,
                    in_=xkT[:, ts(tc_i, 512)].rearrange("(a p) m -> p a m", p=128),
                )
                for dc in range(8):
                    ps = psA.tile([128, 512], dt.float32, tag="mm", name="mm")
                    for kc in range(8):
                        nc.tensor.matmul(
                            ps,
                            lhsT=wk_sb[:, kc, ts(dc, 128)],
                            rhs=xk_sb[:, kc, :],
                            start=(kc == 0),
                            stop=(kc == 7),
                        )
                    stg = stp.tile([128, 512], dt.bfloat16, tag="kstage", name="kstage")
                    nc.scalar.activation(stg, ps, Ident, bias=bk_sb[:, dc, :])
                    nc.sync.dma_start(out=kT_d[ts(dc, 128), ts(tc_i, 512)], in_=stg)

            # ---- P3: V projection (full batch, ones col fused) -> v_d ----
            for tc_i in range(KC):
                xv_sb = xvp.tile([128, 8, 128], dt.bfloat16, tag="xv", name="xv")
                nc.sync.dma_start(
                    out=xv_sb,
                    in_=xvT[:, ts(tc_i, 128)].rearrange("(a p) m -> p a m", p=128),
                )
                for g in range(4):
                    ps = psA.tile([128, 4 * VG], dt.float32, tag="mm", name="mm")
                    for kc in range(8):
                        nc.tensor.matmul(
                            ps,
                            lhsT=xv_sb[:, kc, :],
                            rhs=wv_sb[:, kc, ts(g, 4 * VG)],
                            start=(kc == 0),
                            stop=False,
                        )
                    nc.tensor.matmul(
                        ps,
                        lhsT=ones_sb[:, :],
                        rhs=bv_sb[:, ts(g, 4 * VG)],
                        start=False,
                        stop=True,
                    )
                    stg = stp.tile([128, 4 * VG], dt.bfloat16, tag="vstage", name="vstage")
                    nc.scalar.activation(stg, ps, Copy)
                    nc.sync.dma_start(
                        out=v_d[ts(tc_i, 128), ts(g, 4 * VG)], in_=stg
                    )

            # ---- P4: attention per head ----
            for h in range(H):
                kat = katp.tile([CP, T], dt.bfloat16, tag="kat", name="kat")
                nc.sync.dma_start(out=kat[0:DH, :], in_=kT_d[ds(DH * h, DH), :])
                nc.sync.dma_start(out=kat[DH:CP, :], in_=kfE)
                ps_av = psAV.tile([VG, TQ], dt.float32, tag="av", name="av")
                for kc in range(KC):
                    ps_s = psS.tile([128, TQ], dt.float32, tag="s", name="s")
                    nc.tensor.matmul(
                        ps_s,
                        lhsT=kat[:, ts(kc, 128)],
                        rhs=qat[h],
                        start=True,
                        stop=True,
                    )
                    eh = ep.tile([128, TQ], dt.bfloat16, tag="ehat", name="ehat")
                    nc.scalar.activation(eh, ps_s, Exp)
                    vat = vatp.tile([128, VG], dt.bfloat16, tag="vat", name="vat")
                    nc.sync.dma_start(
                        out=vat, in_=v_d[ts(kc, 128), ds(VG * h, VG)]
                    )
                    nc.tensor.matmul(
                        ps_av,
                        lhsT=vat,
                        rhs=eh,
                        start=(kc == 0),
                        stop=(kc == KC - 1),
                    )
                # unnormalized head output + denominator
                nc.scalar.activation(aot[h], ps_av[0:DH, :], Copy)
                den = denp.tile([VG, TQ], dt.float32, tag="den", name="den")
                nc.scalar.activation(den[DH:VG, :], ps_av[DH:VG, :], Copy)
                nc.sync.dma_start(out=dsum[ds(h, 1), :], in_=den[DH:VG, :])

            # ---- P5: normalization ----
            nc.vector.reciprocal_approx_fast(rcp, dsum)
            nc.vector.tensor_copy(rcp_bf, rcp)
            for h in range(H):
                r1 = rsbp.tile([1, TQ], dt.bfloat16, tag="r1", name="r1")
                nc.sync.dma_start(out=r1, in_=rcp_bf[ds(h, 1), :])
                ps_r = psR.tile([DH, TQ], dt.float32, tag="r", name="r")
                nc.tensor.matmul(
                    ps_r,
                    lhsT=ones_sb[:, 0:DH],
                    rhs=r1,
                    start=True,
                    stop=True,
                )
                r_sb = rsbp.tile([DH, TQ], dt.bfloat16, tag="rsb", name="rsb")
                nc.scalar.activation(r_sb, ps_r, Copy)
                nc.vector.tensor_mul(aot[h], aot[h], r_sb)

            # ---- P6: output projection ----
            for qc in range(4):
                for mc in range(2):
                    ps_y = psA.tile([128, 512], dt.float32, tag="mm", name="mm")
                    for hh in range(H):
                        nc.tensor.matmul(
                            ps_y,
                            lhsT=aot[hh][:, ts(qc, 128)],
                            rhs=wo_sb[hh][:, ds(512 * mc, 512)],
                            start=(hh == 0),
                            stop=False,
                        )
                    nc.tensor.matmul(
                        ps_y,
                        lhsT=ones_sb[:, :],
                        rhs=bo_sb[:, ds(512 * mc, 512)],
                        start=False,
                        stop=True,
                    )
                    ystg = ysp.tile([128, 512], dt.float32, tag="ystage", name="ystage")
                    nc.scalar.activation(ystg, ps_y, Copy)
                    nc.sync.dma_start(
                        out=outE[ts(qc, 128), ts(mc, 512)], in_=ystg
                    )

    nc.compile()
    _state["nc"] = nc
    return nc


def _make_in_maps(inputs):
    _features()
    gs = float(np.float32(inputs["gamma"])) * DH ** -0.5
    delta = float(np.float32(inputs["delta"]))
    ap_ = np.asarray(inputs["alpha_pos"], np.float64)
    an_ = np.asarray(inputs["alpha_neg"], np.float64)

    wqTh = (np.asarray(inputs["Wq"], np.float64).T * gs).astype(BF)
    bqh = (np.asarray(inputs["bq"], np.float64) * gs).astype(np.float32)[:, None]
    wkTh = np.ascontiguousarray(np.asarray(inputs["Wk"]).T).astype(BF)
    bkh = np.asarray(inputs["bk"], np.float32)[:, None]
    woTh = np.ascontiguousarray(np.asarray(inputs["Wo"]).T).astype(BF)
    boh = np.asarray(inputs["bo"], np.float32)[None, :].astype(BF)

    # pre-spaced Wv^T with zero columns for the fused ones column
    wvT = np.asarray(inputs["Wv"], np.float64).T  # [D(in), D(out)]
    wvP = np.zeros((D, H * VG), np.float64)
    bvA = np.zeros((1, H * VG), np.float64)
    for h in range(H):
        wvP[:, VG * h : VG * h + DH] = wvT[:, DH * h : DH * h + DH]
        bvA[0, VG * h : VG * h + DH] = np.asarray(inputs["bv"], np.float64)[
            DH * h : DH * h + DH
        ]
        bvA[0, VG * h + DH] = 1.0
    wvP = wvP.astype(BF)
    bvA = bvA.astype(BF)

    phi, psi = _state["phi"], _state["psi"]
    frac = np.asarray(inputs["frac"], np.float64)

    in_maps = []
    for c in range(NCORES):
        b, p = c // 4, c % 4
        fb = frac[b]
        fq = fb[TQ * p : TQ * (p + 1)]
        kfeat = np.concatenate([_ev(phi, fb), _ev(psi, fb)], 0).astype(BF)
        qfeat = np.zeros((H * AUG, TQ), np.float64)
        for h in range(H):
            a_h = delta * ap_[h] / NGRID
            b_h = -delta * an_[h] / NGRID
            qfeat[AUG * h : AUG * h + R] = a_h * _ev(psi, fq)
            qfeat[AUG * h + R : AUG * (h + 1)] = b_h * _ev(phi, fq)
        qfeat = qfeat.astype(BF)

        xq = np.asarray(inputs["query"])[b, TQ * p : TQ * (p + 1)]
        in_maps.append(
            {
                "xqT": np.ascontiguousarray(xq.T).astype(BF),
                "xkT": np.ascontiguousarray(np.asarray(inputs["key"])[b].T).astype(BF),
                "xvT": np.ascontiguousarray(np.asarray(inputs["value"])[b].T).astype(
                    BF
                ),
                "wqT": wqTh,
                "wkT": wkTh,
                "wvP": wvP,
                "woT": woTh,
                "bq": bqh,
                "bk": bkh,
                "bvA": bvA,
                "bo": boh,
                "kfeat": kfeat,
                "qfeat": qfeat,
            }
        )
    return in_maps


def _run(inputs, trace=False, **kw):
    from concourse.bass_utils import run_bass_kernel_spmd

    nc = _build()
    in_maps = _make_in_maps(inputs)
    res = run_bass_kernel_spmd(
        nc, in_maps, core_ids=list(range(NCORES)), trace=trace, **kw
    )
    out = np.zeros((B, T, D), np.float32)
    for c in range(NCORES):
        b, p = c // 4, c % 4
        out[b, TQ * p : TQ * (p + 1)] = res.results[c]["out"]
    return out, res


def kernel(**inputs):
    out, _ = _run(inputs)
    return out


# revision 15
# speedup vs baseline: 1.0163x; 1.0163x over previous
"""Distributed Trainium2 kernel for CustomMultiHeadAttentionStoich.

Sharding (8 cores): core c = (batch b=c//4, slice p=c%4).
 - Q projection: full, for the core's 512-query slice.
 - K^T projection: head-sharded within the batch group (core computes d-rows
   [256p, 256p+256) for all 2048 keys), then AllGather over the 4-core group
   concatenates along d -> full K^T [1024, 2048] in DRAM.
 - V projection: key-sharded (rows [512p, 512p+512), all heads, with a fused
   ones-column per head for softmax denominators), AllGather along t -> full
   V_aug [2048, 16*65] in DRAM.
 - Attention + out-projection for the core's 512 queries; host concatenates
   the 8 disjoint output slices.

The stoichiometric bias alpha_pos*relu(clamp(fk-fq)) + alpha_neg*min(clamp,0)
is a rank-32 SVD feature expansion injected as extra contraction rows of the
scores matmul (head_dim 64 leaves idle PE rows: zero extra cycles).

Scores are in [key, query] layout; exp is batched over 2-bank PSUM tiles on
ScalarE; softmax denominators come from the V ones-column; normalization is
fused per-head on VectorE (reciprocal_approx_fast + PE broadcast).
"""

import sys

sys.path.insert(0, "/opt/trn_rl_repo")

import numpy as np
import ml_dtypes

BF = ml_dtypes.bfloat16

B, T, D, H, DH = 2, 2048, 1024, 16, 64
NCORES = 8
TQ = 512  # queries per core
R = 16  # SVD rank per clamp-kernel half
AUG = 2 * R
CP = DH + AUG  # contraction rows for the scores matmul
NGRID = 1024  # SVD grid
KC = T // 128  # 16 key chunks
VG = 65  # V columns per head incl. ones column
HG = 4  # heads per core for the K projection

_state = {}


def _features():
    """Rank-R SVD features of rc(x,y)=clip(x-y,0,0.2) on [0,1]^2."""
    if "grid" not in _state:
        g = (np.arange(NGRID) + 0.5) / NGRID
        M = np.clip(g[:, None] - g[None, :], 0.0, 0.2)
        U, S, Vt = np.linalg.svd(M, full_matrices=False)
        sc = np.sqrt(S[:R] * NGRID)
        _state["grid"] = g
        _state["phi"] = (U[:, :R] * sc).astype(np.float64)  # [NGRID, R] phi_j(x)
        _state["psi"] = (Vt[:R].T * sc).astype(np.float64)  # [NGRID, R] psi_j(y)
    return _state["grid"], _state["phi"], _state["psi"]


def _ev(tab, x):
    g = _state["grid"]
    return np.stack([np.interp(x, g, tab[:, j]) for j in range(R)])


def _build():
    if "nc" in _state:
        return _state["nc"]

    import concourse.bass as bass
    import concourse.mybir as mybir
    import concourse.tile as tile
    from concourse import bacc

    dt = mybir.dt
    ts = bass.ts
    ds = bass.ds

    nc = bacc.Bacc(
        "TRN2",
        target_bir_lowering=False,
        debug=False,
        num_devices=NCORES,
    )

    # ---- kernel I/O (per-core shards; host pre-slices) ----
    xqT = nc.dram_tensor("xqT", [D, TQ], dt.bfloat16, kind="ExternalInput").ap()
    xkT = nc.dram_tensor("xkT", [D, T], dt.bfloat16, kind="ExternalInput").ap()
    xvT = nc.dram_tensor("xvT", [D, TQ], dt.bfloat16, kind="ExternalInput").ap()
    wqT = nc.dram_tensor("wqT", [D, D], dt.bfloat16, kind="ExternalInput").ap()
    # per-core head-group slice of Wk^T
    wkT = nc.dram_tensor("wkT", [D, HG * DH], dt.bfloat16, kind="ExternalInput").ap()
    wvP = nc.dram_tensor("wvP", [D, H * VG], dt.bfloat16, kind="ExternalInput").ap()
    woT = nc.dram_tensor("woT", [D, D], dt.bfloat16, kind="ExternalInput").ap()
    bqE = nc.dram_tensor("bq", [D, 1], dt.float32, kind="ExternalInput").ap()
    bkE = nc.dram_tensor("bk", [HG * DH, 1], dt.float32, kind="ExternalInput").ap()
    bvA = nc.dram_tensor("bvA", [1, H * VG], dt.bfloat16, kind="ExternalInput").ap()
    boE = nc.dram_tensor("bo", [1, D], dt.bfloat16, kind="ExternalInput").ap()
    kfE = nc.dram_tensor("kfeat", [AUG, T], dt.bfloat16, kind="ExternalInput").ap()
    qfE = nc.dram_tensor("qfeat", [H * AUG, TQ], dt.bfloat16, kind="ExternalInput").ap()
    outE = nc.dram_tensor("out", [TQ, D], dt.float32, kind="ExternalOutput").ap()
    import os as _os
    _DBG = bool(_os.environ.get("KERNEL_DEBUG"))
    if _DBG:
        dbgK = nc.dram_tensor("dbg_k", [D, T], dt.bfloat16, kind="ExternalOutput").ap()
        dbgV = nc.dram_tensor("dbg_v", [T, H * VG], dt.bfloat16, kind="ExternalOutput").ap()
        dbgE = nc.dram_tensor("dbg_e", [KC * 128, TQ], dt.bfloat16, kind="ExternalOutput").ap()
        dbgVT = nc.dram_tensor("dbg_vt", [128, KC * VG], dt.bfloat16, kind="ExternalOutput").ap()
        dbgDen = nc.dram_tensor("dbg_den", [H, TQ], dt.float32, kind="ExternalOutput").ap()
        dbgU = nc.dram_tensor("dbg_u", [H * DH, TQ], dt.bfloat16, kind="ExternalOutput").ap()
        dbgAo = nc.dram_tensor("dbg_ao", [H * DH, TQ], dt.bfloat16, kind="ExternalOutput").ap()

    Exp = mybir.ActivationFunctionType.Exp
    Copy = mybir.ActivationFunctionType.Copy
    RG = [[0, 1, 2, 3], [4, 5, 6, 7]]
    Bypass = mybir.AluOpType.bypass

    with tile.TileContext(nc) as tc:
        with (
            tc.tile_pool(name="dram", bufs=1, space="DRAM") as dram,
            tc.tile_pool(name="consts", bufs=1) as consts,
            tc.tile_pool(name="kat", bufs=2) as katp,
            tc.tile_pool(name="vtl", bufs=2) as vtlp,
            tc.tile_pool(name="ehat", bufs=3) as ep,
            tc.tile_pool(name="stage", bufs=4) as stp,
            tc.tile_pool(name="rr", bufs=2) as rrp,
            tc.tile_pool(name="ystage", bufs=2) as ysp,
            tc.tile_pool(name="psA", bufs=2, space="PSUM") as psA,
            tc.tile_pool(name="psS", bufs=3, space="PSUM") as psS,
            tc.tile_pool(name="psAV", bufs=2, space="PSUM") as psAV,
        ):
            # DRAM: local shards + gathered full tensors (collective outputs)
            kT_part = dram.tile([HG * DH, T], dt.bfloat16, tag="kT_part", name="kT_part")
            v_part = dram.tile([TQ, H * VG], dt.bfloat16, tag="v_part", name="v_part")
            kT_full = dram.tile([D, T], dt.bfloat16, tag="kT_full", name="kT_full")
            v_full = dram.tile(
                [T, H * VG], dt.bfloat16, tag="v_full", name="v_full"
            )

            # ---- resident constants ----
            wq_sb = consts.tile([128, 8, D], dt.bfloat16, tag="wq", name="wq")
            wk_sb = consts.tile([128, 8, HG * DH], dt.bfloat16, tag="wk", name="wk")
            wv_sb = consts.tile([128, 8, H * VG], dt.bfloat16, tag="wv", name="wv")
            wo_sb = [
                consts.tile([DH, D], dt.bfloat16, tag=f"wo{h}", name=f"wo{h}")
                for h in range(H)
            ]
            xq_sb = consts.tile([128, 8, TQ], dt.bfloat16, tag="xq", name="xq")
            xk_sb = consts.tile([128, 8, T], dt.bfloat16, tag="xk", name="xk")
            xv_sb = consts.tile([128, 8, TQ], dt.bfloat16, tag="xv", name="xv")
            bq_sb = consts.tile([128, 8, 1], dt.float32, tag="bq", name="bq")
            bk_sb = consts.tile([128, 2, 1], dt.float32, tag="bk", name="bk")
            bv_sb = consts.tile([1, H * VG], dt.bfloat16, tag="bv", name="bv")
            bo_sb = consts.tile([1, D], dt.bfloat16, tag="bo", name="bo")
            ones_sb = consts.tile([1, 128], dt.bfloat16, tag="ones", name="ones")
            dsum = consts.tile([H, TQ], dt.float32, tag="dsum", name="dsum")
            rcp = consts.tile([H, TQ], dt.float32, tag="rcp", name="rcp")
            rcp_bf = consts.tile([H, TQ], dt.bfloat16, tag="rcpbf", name="rcpbf")

            for (dst, src) in (
                (wq_sb, wqT),
                (wk_sb, wkT),
                (wv_sb, wvP),
                (xq_sb, xqT),
                (xk_sb, xkT),
                (xv_sb, xvT),
            ):
                nc.sync.dma_start(out=dst, in_=src.rearrange("(a p) m -> p a m", p=128))
            for h in range(H):
                nc.sync.dma_start(out=wo_sb[h], in_=woT[ds(DH * h, DH), :])
            nc.sync.dma_start(out=bq_sb, in_=bqE.rearrange("(a p) o -> p a o", p=128))
            nc.sync.dma_start(out=bk_sb, in_=bkE.rearrange("(a p) o -> p a o", p=128))
            nc.sync.dma_start(out=bv_sb, in_=bvA)
            nc.sync.dma_start(out=bo_sb, in_=boE)
            nc.vector.memset(ones_sb, 1.0)

            qat = [
                consts.tile([CP, TQ], dt.bfloat16, tag=f"qat{h}", name=f"qat{h}")
                for h in range(H)
            ]
            aot = [
                consts.tile([DH, TQ], dt.bfloat16, tag=f"aot{h}", name=f"aot{h}")
                for h in range(H)
            ]

            # ---- P2: K^T projection (head-group shard) -> kT_part -> AllGather
            for dc in range(2):
                for tc_i in range(4):
                    ps = psA.tile([128, 512], dt.float32, tag="mm", name="mmk")
                    for kc in range(8):
                        nc.tensor.matmul(
                            ps,
                            lhsT=wk_sb[:, kc, ts(dc, 128)],
                            rhs=xk_sb[:, kc, ts(tc_i, 512)],
                            start=(kc == 0),
                            stop=(kc == 7),
                        )
                    stg = stp.tile([128, 512], dt.bfloat16, tag="kstage", name="kstage")
                    nc.vector.tensor_scalar_add(stg, ps, bk_sb[:, dc, :])
                    nc.sync.dma_start(out=kT_part[ts(dc, 128), ts(tc_i, 512)], in_=stg)
            nc.gpsimd.collective_compute(
                "AllGather",
                Bypass,
                ins=[kT_part.opt()],
                outs=[kT_full.opt()],
                replica_groups=RG,
            )

            # ---- P3: V projection (key shard, ones fused) -> v_part -> AllGather
            for tc_i in range(4):
                for g in range(4):
                    ps = psA.tile([128, 4 * VG], dt.float32, tag="mm", name="mmv")
                    for kc in range(8):
                        nc.tensor.matmul(
                            ps,
                            lhsT=xv_sb[:, kc, ts(tc_i, 128)],
                            rhs=wv_sb[:, kc, ts(g, 4 * VG)],
                            start=(kc == 0),
                            stop=False,
                        )
                    nc.tensor.matmul(
                        ps,
                        lhsT=ones_sb[:, :],
                        rhs=bv_sb[:, ts(g, 4 * VG)],
                        start=False,
                        stop=True,
                    )
                    stg = stp.tile([128, 4 * VG], dt.bfloat16, tag="vstage", name="vstage")
                    nc.vector.tensor_copy(stg, ps)
                    nc.sync.dma_start(
                        out=v_part[ts(tc_i, 128), ts(g, 4 * VG)], in_=stg
                    )
            nc.gpsimd.collective_compute(
                "AllGather",
                Bypass,
                ins=[v_part.opt()],
                outs=[v_full.opt()],
                replica_groups=RG,
            )

            if _DBG:
                nc.sync.dma_start(out=dbgK, in_=kT_full)
                nc.sync.dma_start(out=dbgV, in_=v_full)

            # ---- P1: Q projection (after K/V so PE warms into attention) ----
            for dc in range(8):
                ps = psA.tile([128, TQ], dt.float32, tag="mm", name="mmq")
                for kc in range(8):
                    nc.tensor.matmul(
                        ps,
                        lhsT=wq_sb[:, kc, ts(dc, 128)],
                        rhs=xq_sb[:, kc, :],
                        start=(kc == 0),
                        stop=(kc == 7),
                    )
                stg = stp.tile([128, TQ], dt.bfloat16, tag="qstage", name="qstage")
                nc.vector.tensor_scalar_add(stg, ps, bq_sb[:, dc, :])
                nc.sync.dma_start(out=qat[2 * dc][0:DH, :], in_=stg[0:DH, :])
                nc.sync.dma_start(out=qat[2 * dc + 1][0:DH, :], in_=stg[DH:128, :])
            for h in range(H):
                nc.sync.dma_start(out=qat[h][DH:CP, :], in_=qfE[ds(AUG * h, AUG), :])

            # ---- P4 + P5: attention per head, fused normalization ----
            for h in range(H):
                kat = katp.tile([CP, T], dt.bfloat16, tag="kat", name="kat")
                nc.sync.dma_start(out=kat[0:DH, :], in_=kT_full[ds(DH * h, DH), :])
                nc.sync.dma_start(out=kat[DH:CP, :], in_=kfE)
                vtl = vtlp.tile([128, KC, VG], dt.bfloat16, tag="vtl", name="vtl")
                nc.sync.dma_start(
                    out=vtl,
                    in_=v_full[:, ds(VG * h, VG)].rearrange(
                        "(a p) m -> p a m", p=128
                    ),
                )
                ps_av = psAV.tile([VG, TQ], dt.float32, tag="av", name="av")
                for kc in range(KC):
                    ps_s = psS.tile([128, TQ], dt.float32, tag="s", name="s")
                    nc.tensor.matmul(
                        ps_s,
                        lhsT=kat[:, ts(kc, 128)],
                        rhs=qat[h],
                        start=True,
                        stop=True,
                    )
                    eh = ep.tile([128, TQ], dt.bfloat16, tag="ehat", name="ehat")
                    nc.scalar.activation(eh, ps_s, Exp)
                    if _DBG and h == 0:
                        nc.sync.dma_start(out=dbgE[ts(kc, 128), :], in_=eh)
                    nc.tensor.matmul(
                        ps_av,
                        lhsT=vtl[:, kc, :],
                        rhs=eh,
                        start=(kc == 0),
                        stop=(kc == KC - 1),
                    )
                # unnormalized head output + denominator (v1-proven pattern)
                nc.scalar.activation(aot[h], ps_av[0:DH, :], Copy)
                if _DBG and h == 0:
                    nc.sync.dma_start(out=dbgVT, in_=vtl.rearrange("p a m -> p (a m)"))
                if _DBG:
                    nc.sync.dma_start(out=dbgU[ds(DH * h, DH), :], in_=aot[h])
                den = rrp.tile([VG, TQ], dt.float32, tag="den", name="den")
                nc.scalar.activation(den[DH:VG, :], ps_av[DH:VG, :], Copy)
                nc.sync.dma_start(out=dsum[ds(h, 1), :], in_=den[DH:VG, :])
                if _DBG:
                    nc.sync.dma_start(out=dbgDen[ds(h, 1), :], in_=den[DH:VG, :])

            # ---- P5: batched normalization ----
            nc.vector.reciprocal_approx_fast(rcp, dsum)
            nc.vector.tensor_copy(rcp_bf, rcp)
            for h in range(H):
                r1 = rrp.tile([1, TQ], dt.bfloat16, tag="r1", name="r1")
                nc.sync.dma_start(out=r1, in_=rcp_bf[ds(h, 1), :])
                ps_r = psA.tile([DH, TQ], dt.float32, tag="mm", name="mmr")
                nc.tensor.matmul(
                    ps_r, lhsT=ones_sb[:, 0:DH], rhs=r1, start=True, stop=True
                )
                r_sb = rrp.tile([DH, TQ], dt.bfloat16, tag="rsb", name="rsb")
                nc.scalar.activation(r_sb, ps_r, Copy)
                nc.vector.tensor_mul(aot[h], aot[h], r_sb)
                if _DBG:
                    nc.sync.dma_start(out=dbgAo[ds(DH * h, DH), :], in_=aot[h])

            # ---- P6: output projection ----
            for qc in range(4):
                for mc in range(2):
                    ps_y = psA.tile([128, 512], dt.float32, tag="mm", name="mmy")
                    for hh in range(H):
                        nc.tensor.matmul(
                            ps_y,
                            lhsT=aot[hh][:, ts(qc, 128)],
                            rhs=wo_sb[hh][:, ds(512 * mc, 512)],
                            start=(hh == 0),
                            stop=False,
                        )
                    nc.tensor.matmul(
                        ps_y,
                        lhsT=ones_sb[:, :],
                        rhs=bo_sb[:, ds(512 * mc, 512)],
                        start=False,
                        stop=True,
                    )
                    ystg = ysp.tile([128, 512], dt.float32, tag="ystage", name="ystage")
                    nc.vector.tensor_copy(ystg, ps_y)
                    nc.sync.dma_start(out=outE[ts(qc, 128), ts(mc, 512)], in_=ystg)

    nc.compile()
    _state["nc"] = nc
    return nc


def _make_in_maps(inputs):
    _features()
    gs = float(np.float32(inputs["gamma"])) * DH ** -0.5
    delta = float(np.float32(inputs["delta"]))
    ap_ = np.asarray(inputs["alpha_pos"], np.float64)
    an_ = np.asarray(inputs["alpha_neg"], np.float64)

    wqTh = (np.asarray(inputs["Wq"], np.float64).T * gs).astype(BF)
    bqh = (np.asarray(inputs["bq"], np.float64) * gs).astype(np.float32)[:, None]
    wkT_full = np.ascontiguousarray(np.asarray(inputs["Wk"]).T)  # [D_in, D_out]
    bk_full = np.asarray(inputs["bk"], np.float32)
    woTh = np.ascontiguousarray(np.asarray(inputs["Wo"]).T).astype(BF)
    boh = np.asarray(inputs["bo"], np.float32)[None, :].astype(BF)

    wvT = np.asarray(inputs["Wv"], np.float64).T
    wvP = np.zeros((D, H * VG), np.float64)
    bvA = np.zeros((1, H * VG), np.float64)
    for h in range(H):
        wvP[:, VG * h : VG * h + DH] = wvT[:, DH * h : DH * h + DH]
        bvA[0, VG * h : VG * h + DH] = np.asarray(inputs["bv"], np.float64)[
            DH * h : DH * h + DH
        ]
        bvA[0, VG * h + DH] = 1.0
    wvP = wvP.astype(BF)
    bvA = bvA.astype(BF)

    phi, psi = _state["phi"], _state["psi"]
    frac = np.asarray(inputs["frac"], np.float64)

    in_maps = []
    for c in range(NCORES):
        b, p = c // 4, c % 4
        fb = frac[b]
        fq = fb[TQ * p : TQ * (p + 1)]
        kfeat = np.concatenate([_ev(phi, fb), _ev(psi, fb)], 0).astype(BF)
        qfeat = np.zeros((H * AUG, TQ), np.float64)
        for h in range(H):
            a_h = delta * ap_[h] / NGRID
            b_h = -delta * an_[h] / NGRID
            qfeat[AUG * h : AUG * h + R] = a_h * _ev(psi, fq)
            qfeat[AUG * h + R : AUG * (h + 1)] = b_h * _ev(phi, fq)
        qfeat = qfeat.astype(BF)

        xq = np.asarray(inputs["query"])[b, TQ * p : TQ * (p + 1)]
        xv = np.asarray(inputs["value"])[b, TQ * p : TQ * (p + 1)]
        in_maps.append(
            {
                "xqT": np.ascontiguousarray(xq.T).astype(BF),
                "xkT": np.ascontiguousarray(np.asarray(inputs["key"])[b].T).astype(BF),
                "xvT": np.ascontiguousarray(xv.T).astype(BF),
                "wqT": wqTh,
                "wkT": np.ascontiguousarray(
                    wkT_full[:, 256 * p : 256 * (p + 1)]
                ).astype(BF),
                "wvP": wvP,
                "woT": woTh,
                "bq": bqh,
                "bk": np.ascontiguousarray(bk_full[256 * p : 256 * (p + 1)])[:, None],
                "bvA": bvA,
                "bo": boh,
                "kfeat": kfeat,
                "qfeat": qfeat,
            }
        )
    return in_maps


def _run(inputs, trace=False, **kw):
    from concourse.bass_utils import run_bass_kernel_spmd

    nc = _build()
    in_maps = _make_in_maps(inputs)
    res = run_bass_kernel_spmd(
        nc, in_maps, core_ids=list(range(NCORES)), trace=trace, **kw
    )
    out = np.zeros((B, T, D), np.float32)
    for c in range(NCORES):
        b, p = c // 4, c % 4
        out[b, TQ * p : TQ * (p + 1)] = res.results[c]["out"]
    return out, res


def kernel(**inputs):
    out, _ = _run(inputs)
    return out
